# revision 26
# baseline (speedup 1.0000x reference)
"""EntNet forward kernel for 8 Trainium2 NeuronCores (Bass/Tile).

Math note: in the reference, the gated memory is
    mem = memory_nodes * (1 + sigmoid(...))
followed by per-column L2 normalization.  Since (1 + sigmoid(x)) > 0 is a
per-column positive scalar, it cancels exactly in the normalization, so the
gate g — and with it s_in, F_i, input, keys, U, V, W, a_mem (cand is dead in
the source already) — does not affect the output.  Live computation:

    s_q = F_q @ query[0]                         # [D]
    mn  = memory_nodes / max(||col||_2, 1e-12)   # [D, M] column-normalized
    p   = softmax(s_q^T @ mn)                    # [1, M]
    u   = mn @ p^T                               # [D]
    y   = R @ prelu(s_q + H @ u, a_out)          # [D, 1]

Sharding: D is row-sharded over 8 cores (rows_c = 512c:512c+512).  Each core
streams its shards of F_q^T, mem, mem^T (row shards), H^T (H column shard),
R^T (R row shard) — ~36 MiB/core vs 608 MiB of live bytes single-core.
Collectives: an early AllReduce of the column sum-of-squares (also absorbs
the cold-start cost of the collective machinery), a second 4 KB AllReduce of
the r = mem^T s_q partials, and a 16 KB AllReduce of z = s_q + H@u partials.

Precision: f32 PE matvecs measure ~430 ns per 128x128 weight tile on trn2,
while bf16 weight-stationary tiles pipeline at ~27 ns.  All matrices are
shipped as bf16 (hi, lo) splits — X = hi + lo with hi = bf16(X),
lo = bf16(X - hi) — and each matvec computes hi@vhi + hi@vlo + lo@vhi in
f32 PSUM (the dropped lo@lo term is O(2^-18) relative).  Same DMA bytes as
f32, ~1e-5 relative output error, >5x PE speedup.

Vectors are partition-major throughout: v[128i + p] <-> tile[p, i].
"""

import sys
import numpy as np

for _p in ("/root/.axon_site/_ro/trn_rl_repo", "/opt/trn_rl_repo"):
    if _p not in sys.path:
        sys.path.append(_p)

D, M, L = 4096, 1024, 8192
N_CORES = 8

_CACHE = {}


def _build_module(n_cores, d, m, l):  # noqa: E741
    import concourse.bacc as bacc
    import concourse.tile as tile
    import concourse.tile as tile_mod
    import concourse.mybir as mybir
    import concourse.bass_isa as bass_isa

    f32 = mybir.dt.float32
    bf16 = mybir.dt.bfloat16
    DL = d // n_cores      # local rows of D
    KD = DL // 128         # local d chunks
    KM = m // 128          # m chunks
    KZ = d // 128          # global d chunks
    KL = l // 128          # l chunks
    AF = mybir.ActivationFunctionType
    ADD = mybir.AluOpType.add
    rg = [list(range(n_cores))]

    # F_q^T hi|lo streamed in chunks of FQ_N l-tiles (8 -> 1 MiB at full size)
    FQ_N = min(8, KL)
    assert (2 * KL) % FQ_N == 0
    RT_N = min(8, KZ)
    assert (2 * KZ) % RT_N == 0

    nc = bacc.Bacc("TRN2", target_bir_lowering=False, debug=False,
                   enable_asserts=False, num_devices=n_cores)

    # hi|lo packed along the leading axis: rows [0:n] = hi, [n:2n] = lo
    FQCH = (2 * KL) // FQ_N
    fqT_in = nc.dram_tensor("fqT", [FQCH * 128, FQ_N * DL], bf16,
                            kind="ExternalInput")
    q2d_in = nc.dram_tensor("q2d", [128, 2 * KL], bf16, kind="ExternalInput")
    memd_in = nc.dram_tensor("memd", [128, 2 * KD * m], bf16, kind="ExternalInput")
    memT_in = nc.dram_tensor("memT", [128, 2 * KM * DL], bf16, kind="ExternalInput")
    hT_in = nc.dram_tensor("hT", [128, 2 * KD * d], bf16, kind="ExternalInput")
    rT_in = nc.dram_tensor("rT", [128, 2 * KZ * DL], bf16, kind="ExternalInput")
    ab_in = nc.dram_tensor("ab", [128, 1], f32, kind="ExternalInput")
    mask_in = nc.dram_tensor("mask", [128, KZ], f32, kind="ExternalInput")
    ssf_in = nc.dram_tensor("ssf", [128, KM], f32, kind="ExternalInput")
    y_out = nc.dram_tensor("y", [128, KD], f32, kind="ExternalOutput")
    hw_out = nc.dram_tensor("hw", [1, 8], f32, kind="ExternalOutput")

    with tile.TileContext(nc) as tc:
        with (
            tc.tile_pool(name="consts", bufs=1) as consts,
            tc.tile_pool(name="fq", bufs=3) as fqp,
            tc.tile_pool(name="mem", bufs=1) as memp,
            tc.tile_pool(name="big", bufs=1) as bigp,
            tc.tile_pool(name="sm", bufs=1) as smp,
            tc.tile_pool(name="scr", bufs=2) as scrp,
            tc.tile_pool(name="ps_sq", bufs=1, space="PSUM") as ps_sq,
            tc.tile_pool(name="ps_r", bufs=1, space="PSUM") as ps_r,
            tc.tile_pool(name="ps_u", bufs=1, space="PSUM") as ps_u,
            tc.tile_pool(name="ps_z", bufs=1, space="PSUM") as ps_z,
            tc.tile_pool(name="ps_y", bufs=1, space="PSUM") as ps_y,
            tc.tile_pool(name="ps_h", bufs=1, space="PSUM") as ps_h,
            tc.tile_pool(name="dram", bufs=1, space="DRAM") as dram,
        ):
            # ---- ACT table warmup (Sqrt/Exp/Relu load ~1.3us each lazily) ----
            warm = consts.tile([1, 1], f32)
            nc.gpsimd.memset(warm[:], 1.0)
            w2 = consts.tile([1, 1], f32)
            nc.scalar.activation(w2[:], warm[:], AF.Square)
            nc.scalar.activation(w2[:], warm[:], AF.Sqrt)
            nc.scalar.activation(w2[:], warm[:], AF.Exp)
            nc.scalar.activation(w2[:], warm[:], AF.Relu)

            # ---- small constants ----
            q2d = consts.tile([128, 2 * KL], bf16)
            nc.gpsimd.dma_start(q2d[:], q2d_in[:])
            ab = consts.tile([128, 1], f32)
            nc.gpsimd.dma_start(ab[:], ab_in[:])
            mask = consts.tile([128, KZ], f32)
            nc.gpsimd.dma_start(mask[:], mask_in[:])
            ssf = consts.tile([128, KM], f32)
            nc.gpsimd.dma_start(ssf[:], ssf_in[:])

            # ---- dummy collective: absorbs the cold ncfw/CC start while the
            # stream runs; nothing on the critical path consumes it ----
            wsrc = consts.tile([128, 4], f32)
            nc.gpsimd.memset(wsrc[:], 0.0)
            cw_i = dram.tile([128, 4], f32)
            cw_o = dram.tile([128, 4], f32)
            nc.gpsimd.dma_start(cw_i[:], wsrc[:])
            nc.gpsimd.collective_compute(
                "AllReduce", ADD, replica_groups=rg,
                ins=[cw_i[:].opt()], outs=[cw_o[:].opt()])


            def split_hl(src_f32, k, name):
                """f32 [128, k] -> bf16 [128, 2k] = [hi | lo]."""
                hl = smp.tile([128, 2 * k], bf16, name=name)
                nc.vector.tensor_copy(hl[:, 0:k], src_f32)
                hi_f = smp.tile([128, k], f32, name=name + "_hf")
                nc.vector.tensor_copy(hi_f[:], hl[:, 0:k])
                lo_f = smp.tile([128, k], f32, name=name + "_lf")
                nc.vector.tensor_sub(lo_f[:], src_f32, hi_f[:])
                nc.vector.tensor_copy(hl[:, k:2 * k], lo_f[:])
                return hl

            memd = memp.tile([128, 2 * KD, m], bf16)
            memT = memp.tile([128, 2 * KM, DL], bf16)

            # ---- phase 1: s_q = F_q @ query, streaming F_q^T (hi then lo) ----
            psq = ps_sq.tile([128, KD], f32)
            fqT_r = fqT_in[:].rearrange("(i p) x -> p i x", p=128)
            n_mm = 0
            N_MM_TOT = 2 * KL * KD + KL * KD
            mem_inject = min(4, (2 * KL) // FQ_N - 1)
            for i in range((2 * KL) // FQ_N):
                if i == mem_inject:
                    # mem shards ride the same ring, early enough for ss/r
                    nc.sync.dma_start(
                        memT[:].rearrange("p a b -> p (a b)"), memT_in[:])
                    nc.sync.dma_start(
                        memd[:].rearrange("p a b -> p (a b)"), memd_in[:])
                fq_t = fqp.tile([128, FQ_N, DL], bf16)
                nc.sync.dma_start(
                    fq_t[:].rearrange("p a b -> p (a b)"), fqT_r[:, i, :])
                for j in range(FQ_N):
                    n = FQ_N * i + j
                    is_hi = n < KL
                    nl = n if is_hi else n - KL
                    # hi tile: hi@qhi + hi@qlo ; lo tile: lo@qhi
                    rhs_cols = ([nl, KL + nl] if is_hi else [nl])
                    for dt in range(KD):
                        for col in rhs_cols:
                            nc.tensor.matmul(
                                psq[:, dt:dt + 1],
                                fq_t[:, j, 128 * dt:128 * (dt + 1)],
                                q2d[:, col:col + 1],
                                start=(n_mm == 0), stop=(n_mm == N_MM_TOT - 1),
                            )
                            n_mm += 1
            s_q = smp.tile([128, KD], f32)
            nc.vector.tensor_copy(s_q[:], psq[:])
            sq_hl = split_hl(s_q[:], KD, "sq_hl")

            # H^T and R^T fully resident via two large sync-ring DMAs
            hT = bigp.tile([128, 2 * KD, d], bf16)
            nc.sync.dma_start(hT[:].rearrange("p a b -> p (a b)"), hT_in[:])
            rT = bigp.tile([128, 2 * KZ, DL], bf16)
            nc.sync.dma_start(rT[:].rearrange("p a b -> p (a b)"), rT_in[:])

            # ---- r = mem^T @ s_q (local-d partial) ----
            pr = ps_r.tile([128, KM], f32)
            n_mm = 0
            N_MM_TOT = 3 * KM * KD
            for mt in range(KM):
                for kc in range(KD):
                    for (bw, col) in ((kc, 0), (kc, KD), (KD + kc, 0)):
                        last_r_mm = nc.tensor.matmul(
                            pr[:, mt:mt + 1],
                            memd[:, bw, 128 * mt:128 * (mt + 1)],
                            sq_hl[:, col + kc:col + kc + 1],
                            start=(n_mm == 0), stop=(n_mm == N_MM_TOT - 1),
                        )
                        n_mm += 1
            r_sb = smp.tile([128, KM], f32)
            r_copy = nc.vector.tensor_copy(r_sb[:], pr[:])

            # PE heater: keep HAM warm through the r-AllReduce wait
            HW_N = min(512, DL)
            ph = ps_h.tile([1, HW_N], f32)
            h1 = []
            for k in range(30):
                h1.append(nc.tensor.matmul(
                    ph[:, :], memT[:, 0, k:k + 1], memT[:, 0, 0:HW_N],
                    start=(k == 0), stop=(k == 29)))
            tile_mod.add_dep_helper(h1[0].ins, last_r_mm.ins, sync=False,
                                    reason="heater after r")

            # ---- AllReduce: r ----
            cr_i = dram.tile([128, KM], f32)
            cr_o = dram.tile([128, KM], f32)
            nc.gpsimd.dma_start(cr_i[:], r_sb[:])
            nc.gpsimd.collective_compute(
                "AllReduce", ADD, replica_groups=rg,
                ins=[cr_i[:].opt()], outs=[cr_o[:].opt()])
            rf = smp.tile([128, KM], f32)
            nc.gpsimd.dma_start(rf[:], cr_o[:])

            # ---- softmax (partition-major), fold 1/denom into p ----
            dn = smp.tile([128, KM], f32)
            nc.scalar.activation(dn[:], ssf[:], AF.Sqrt)
            dnm = smp.tile([128, KM], f32)
            dnm_op = nc.vector.tensor_scalar_max(dnm[:], dn[:], 1e-12)
            tile_mod.add_dep_helper(dnm_op.ins, r_copy.ins, sync=False,
                                    reason="softmax DVE after s_q/r path")
            rdn = smp.tile([128, KM], f32)
            nc.vector.reciprocal(rdn[:], dnm[:])
            t = smp.tile([128, KM], f32)
            nc.vector.tensor_mul(t[:], rf[:], rdn[:])
            tm = smp.tile([128, 1], f32)
            nc.vector.tensor_reduce(tm[:], t[:], mybir.AxisListType.X,
                                    mybir.AluOpType.max)
            tmb = smp.tile([128, 1], f32)
            nc.gpsimd.partition_all_reduce(tmb[:], tm[:], 128,
                                           bass_isa.ReduceOp.max)
            negmx = smp.tile([128, 1], f32)
            nc.vector.tensor_scalar_mul(negmx[:], tmb[:], -1.0)
            e = smp.tile([128, KM], f32)
            esum = smp.tile([128, 1], f32)
            nc.scalar.activation(e[:], t[:], AF.Exp, bias=negmx[:],
                                 accum_out=esum[:])
            esb = smp.tile([128, 1], f32)
            nc.gpsimd.partition_all_reduce(esb[:], esum[:], 128,
                                           bass_isa.ReduceOp.add)
            sd = smp.tile([128, KM], f32)
            nc.vector.tensor_scalar_mul(sd[:], dnm[:], esb[:])
            rsd = smp.tile([128, KM], f32)
            nc.vector.reciprocal(rsd[:], sd[:])
            pt = smp.tile([128, KM], f32)
            nc.vector.tensor_mul(pt[:], e[:], rsd[:])
            pt_hl = split_hl(pt[:], KM, "pt_hl")

            # ---- u = mem @ (p/denom), local rows ----
            pu = ps_u.tile([128, KD], f32)
            n_mm = 0
            N_MM_TOT = 3 * KD * KM
            for dt in range(KD):
                for kc in range(KM):
                    for (bw, col) in ((kc, 0), (kc, KM), (KM + kc, 0)):
                        nc.tensor.matmul(
                            pu[:, dt:dt + 1],
                            memT[:, bw, 128 * dt:128 * (dt + 1)],
                            pt_hl[:, col + kc:col + kc + 1],
                            start=(n_mm == 0), stop=(n_mm == N_MM_TOT - 1),
                        )
                        n_mm += 1
            u_sb = smp.tile([128, KD], f32)
            nc.vector.tensor_copy(u_sb[:], pu[:])
            u_hl = split_hl(u_sb[:], KD, "u_hl")

            # ---- z partial = H[:, cols_c] @ u_c (full-D, partition-major) ----
            pz = ps_z.tile([128, KZ], f32)
            n_mm = 0
            N_MM_TOT = 3 * KD * KZ
            half_z_mm = None
            for dt in range(KZ):
                for kc in range(KD):
                    for (bw, col) in ((kc, 0), (kc, KD), (KD + kc, 0)):
                        last_z_mm = nc.tensor.matmul(
                            pz[:, dt:dt + 1],
                            hT[:, bw, 128 * dt:128 * (dt + 1)],
                            u_hl[:, col + kc:col + kc + 1],
                            start=(n_mm == 0), stop=(n_mm == N_MM_TOT - 1),
                        )
                        n_mm += 1
                if dt == KZ // 2 - 1:
                    half_z_mm = last_z_mm
            h2 = []
            for k in range(24):
                h2.append(nc.tensor.matmul(
                    ph[:, :], memT[:, 0, k:k + 1], memT[:, 0, 0:HW_N],
                    start=(k == 0), stop=(k == 23)))
            tile_mod.add_dep_helper(h2[0].ins, last_z_mm.ins, sync=False,
                                    reason="heater after z")

            # place this core's s_q shard via the per-core mask
            sqm = smp.tile([128, KZ], f32)
            nc.vector.tensor_tensor(
                sqm[:].rearrange("p (r k) -> p r k", k=KD),
                s_q[:].unsqueeze(1).broadcast_to([128, KZ // KD, KD]),
                mask[:].rearrange("p (r k) -> p r k", k=KD),
                mybir.AluOpType.mult,
            )

            # ---- z AllReduce in two pipelined halves: y's first half of
            # j-chunks runs while the second half's AllReduce is in flight
            HZ = KZ // 2
            ar2 = smp.tile([128, KZ], f32)
            c2i = [dram.tile([128, HZ], f32, name=f"c2i{h}") for h in range(2)]
            c2o = [dram.tile([128, HZ], f32, name=f"c2o{h}") for h in range(2)]
            zf = smp.tile([128, KZ], f32)
            pzz = smp.tile([128, KZ], f32)
            pz_hl = smp.tile([128, 2 * KZ], bf16)
            py = ps_y.tile([128, KD], f32)
            n_mm = 0
            N_MM_TOT = 3 * KZ * KD
            for h in range(2):
                sl = slice(HZ * h, HZ * (h + 1))
                nc.vector.tensor_add(ar2[:, sl], sqm[:, sl], pz[:, sl])
                nc.gpsimd.dma_start(c2i[h][:], ar2[:, sl])
                nc.gpsimd.collective_compute(
                    "AllReduce", ADD, replica_groups=rg,
                    ins=[c2i[h][:].opt()], outs=[c2o[h][:].opt()])
                nc.gpsimd.dma_start(zf[:, sl], c2o[h][:])
                # prelu(z) = relu(z) + a * (z - relu(z)) on this half
                pos = smp.tile([128, HZ], f32, name=f"pos{h}")
                nc.scalar.activation(pos[:], zf[:, sl], AF.Relu)
                neg = smp.tile([128, HZ], f32, name=f"neg{h}")
                nc.vector.tensor_sub(neg[:], zf[:, sl], pos[:])
                negs = smp.tile([128, HZ], f32, name=f"negs{h}")
                nc.vector.tensor_scalar_mul(negs[:], neg[:], ab[:])
                nc.vector.tensor_add(pzz[:, sl], pos[:], negs[:])
                # split this half into bf16 hi|lo columns of pz_hl
                hi_f = smp.tile([128, HZ], f32, name=f"pzhf{h}")
                nc.vector.tensor_copy(pz_hl[:, sl], pzz[:, sl])
                nc.vector.tensor_copy(hi_f[:], pz_hl[:, sl])
                lo_f = smp.tile([128, HZ], f32, name=f"pzlf{h}")
                nc.vector.tensor_sub(lo_f[:], pzz[:, sl], hi_f[:])
                nc.vector.tensor_copy(pz_hl[:, KZ + HZ * h:KZ + HZ * (h + 1)],
                                      lo_f[:])
                # y MMs for this half's j-chunks (hi and lo R^T tiles)
                for kc in range(HZ * h, HZ * (h + 1)):
                    for (n, col) in ((kc, kc), (kc, KZ + kc), (KZ + kc, kc)):
                        for dt in range(KD):
                            nc.tensor.matmul(
                                py[:, dt:dt + 1],
                                rT[:, n, 128 * dt:128 * (dt + 1)],
                                pz_hl[:, col:col + 1],
                                start=(n_mm == 0), stop=(n_mm == N_MM_TOT - 1),
                            )
                            n_mm += 1

            y_sb = smp.tile([128, KD], f32)
            y_copy = nc.vector.tensor_copy(y_sb[:], py[:])
            nc.gpsimd.dma_start(y_out[:], y_sb[:])
            # dummy-AR readback last: its completion wait must not block the
            # real collectives' bounces/doorbells on the gpsimd queue
            wsb = smp.tile([128, 4], f32)
            nc.gpsimd.dma_start(wsb[:], cw_o[:])

            # consume heater + warmup results (anti-DCE) via dummy output.
            # Pin these behind the y copy so the scheduler cannot hoist the
            # ssf-dependent copy ahead of the s_q copy on the DVE (that
            # inversion idles the PE ~40us waiting on the ss AllReduce).
            hw_sb = smp.tile([1, 8], f32)
            hw0 = nc.vector.memset(hw_sb[:], 0.0)
            hw1 = nc.vector.tensor_copy(hw_sb[:, 0:4], ph[0:1, 0:4])
            hw2 = nc.vector.tensor_copy(hw_sb[:, 4:8], wsb[0:1, 0:4])
            for hw_op in (hw0, hw1, hw2):
                tile_mod.add_dep_helper(hw_op.ins, y_copy.ins, sync=False,
                                        reason="anti-DCE copies last on DVE")
            nc.gpsimd.dma_start(hw_out[:], hw_sb[:])

    nc.compile()
    return nc


def _get_module(n_cores=N_CORES, d=D, m=M, l=L):  # noqa: E741
    key = (n_cores, d, m, l)
    if key not in _CACHE:
        _CACHE[key] = _build_module(n_cores, d, m, l)
    return _CACHE[key]


def _hl(x):
    """f32 array -> bf16 [hi; lo] stacked along axis 0."""
    import ml_dtypes
    bf = ml_dtypes.bfloat16
    hi = x.astype(bf)
    lo = (x - hi.astype(np.float32)).astype(bf)
    return np.concatenate([hi, lo], axis=0)


def _pack(x, group):
    """[n*128, e] -> [128, ...] per-partition-contiguous: rows grouped into
    chunks of `group` 128-row tiles laid side by side along the free dim."""
    n128, e = x.shape
    n = n128 // 128
    assert n % group == 0
    return np.ascontiguousarray(
        x.reshape(n // group, group, 128, e).transpose(0, 2, 1, 3)
    ).reshape((n // group) * 128, group * e)


def _make_in_maps(n_cores, d, m, l, F_q, query, memory_nodes, H, R, a_out):  # noqa: E741
    f32 = np.float32
    DL = d // n_cores
    KZ = d // 128
    KD = DL // 128
    KL = l // 128
    m_ = m
    q2d = np.ascontiguousarray(query.reshape(KL, 128).T).astype(f32, copy=False)
    ss_full = (memory_nodes.astype(np.float64)**2).sum(axis=0).astype(f32)
    ssf_pm = np.ascontiguousarray(ss_full.reshape(m // 128, 128).T)
    q2d_hl = np.concatenate([_hl(q2d)[:128], _hl(q2d)[128:]], axis=1)
    in_maps = []
    for c in range(n_cores):
        rows = slice(DL * c, DL * (c + 1))
        mask = np.zeros((128, KZ), f32)
        mask[:, KD * c:KD * (c + 1)] = 1.0
        FQ_N = min(8, KL)
        in_maps.append({
            "ssf": ssf_pm,
            "fqT": _pack(_hl(np.ascontiguousarray(F_q[rows].T)), FQ_N),
            "q2d": q2d_hl,
            "memd": _pack(_hl(np.ascontiguousarray(memory_nodes[rows])),
                          2 * (DL // 128)),
            "memT": _pack(_hl(np.ascontiguousarray(memory_nodes[rows].T)),
                          2 * (m // 128)),
            "hT": _pack(_hl(np.ascontiguousarray(H[:, rows].T)),
                        2 * (DL // 128)),
            "rT": _pack(_hl(np.ascontiguousarray(R[rows].T)),
                        2 * (d // 128)),
            "ab": np.full((128, 1), a_out, f32),
            "mask": mask,
        })
    return in_maps


class _PjrtRunner:
    """Cached jit(shard_map(bass_exec)) so repeat kernel() calls skip
    retracing/recompiling (bass_utils.run_bass_kernel_spmd rebuilds the jit
    closure every call)."""

    def __init__(self, nc, n_cores):
        import jax
        from jax.sharding import Mesh, PartitionSpec
        from jax.experimental.shard_map import shard_map
        from concourse import bass2jax
        import concourse.mybir as mybir

        bass2jax.install_neuronx_cc_hook()
        self.n_cores = n_cores
        part_name = (nc.partition_id_tensor.name
                     if nc.partition_id_tensor else None)
        in_names, out_names, out_avals = [], [], []
        for alloc in nc.m.functions[0].allocations:
            if not isinstance(alloc, mybir.MemoryLocationSet):
                continue
            name = alloc.memorylocations[0].name
            if alloc.kind == "ExternalInput":
                if name != part_name:
                    in_names.append(name)
            elif alloc.kind == "ExternalOutput":
                out_names.append(name)
                out_avals.append(jax.core.ShapedArray(
                    tuple(alloc.tensor_shape), mybir.dt.np(alloc.dtype)))
        self.in_names, self.out_names, self.out_avals = in_names, out_names, out_avals
        n_params = len(in_names)
        self.zero_outs = [np.zeros(a.shape, a.dtype) for a in out_avals]
        all_in_names = tuple(in_names + out_names)
        if part_name is not None:
            all_in_names = all_in_names + (part_name,)

        def _body(*args):
            operands = list(args)
            if part_name is not None:
                operands.append(bass2jax.partition_id_tensor())
            outs = bass2jax._bass_exec_p.bind(
                *operands,
                out_avals=tuple(out_avals),
                in_names=all_in_names,
                out_names=tuple(out_names),
                lowering_input_output_aliases=(),
                sim_require_finite=True,
                sim_require_nnan=True,
                nc=nc,
            )
            return tuple(outs)

        devices = jax.devices()[:n_cores]
        mesh = Mesh(np.asarray(devices), ("core",))
        n_out = len(out_names)
        self._fn = jax.jit(
            shard_map(
                _body, mesh=mesh,
                in_specs=(PartitionSpec("core"),) * (n_params + n_out),
                out_specs=(PartitionSpec("core"),) * n_out,
                check_rep=False,
            ),
            keep_unused=True,
        )

    def __call__(self, in_maps):
        n = self.n_cores
        concat_in = [
            np.concatenate([in_maps[c][name] for c in range(n)], axis=0)
            for name in self.in_names
        ]
        concat_zeros = [
            np.zeros((n * z.shape[0], *z.shape[1:]), z.dtype)
            for z in self.zero_outs
        ]
        out_arrs = self._fn(*concat_in, *concat_zeros)
        return [
            {name: np.asarray(out_arrs[i]).reshape(n, *self.out_avals[i].shape)[c]
             for i, name in enumerate(self.out_names)}
            for c in range(n)
        ]


_RUNNER = {}


def _get_runner():
    if "r" not in _RUNNER:
        _RUNNER["r"] = _PjrtRunner(_get_module(), N_CORES)
    return _RUNNER["r"]


def kernel(**inputs):
    f32 = np.float32
    F_q = np.asarray(inputs["F_q"], f32)
    query = np.asarray(inputs["query"], f32).reshape(-1)
    memory_nodes = np.asarray(inputs["memory_nodes"], f32)
    H = np.asarray(inputs["H"], f32)
    R = np.asarray(inputs["R"], f32)
    a_out = float(np.asarray(inputs["a_out"]).reshape(-1)[0])

    in_maps = _make_in_maps(N_CORES, D, M, L, F_q, query, memory_nodes,
                            H, R, a_out)
    results = _get_runner()(in_maps)
    y = np.concatenate(
        [np.ascontiguousarray(results[c]["y"].T).reshape(-1)
         for c in range(N_CORES)])
    return y.reshape(D, 1).astype(f32)
